# revision 1
# baseline (speedup 1.0000x reference)
"""CSAA (criss-cross axial attention) Trainium2 kernel.

Sharding: pure data parallel — batch element b -> NeuronCore b (B=8 on 8 cores).

Per-core pipeline (R=128, CIN=COUT=256, H=W=128, HW=16384), per batch element:
  x [CIN, HW] --f32r matmul--> Z [r, (h,w)] bf16            (+br, exact)
  Z --PE transpose per w-->    Zp [h, w*128+r] bf16
  width branch (contract h):   qkv -> attention -> Wp [w, h*128+r]  (+bvw)
  height branch (contract w):  qkv -> attention -> H2 [r, h*128+w]  (+bvh, /sums)
  restore:                     Y [co, (h,w)] f32             (+bo)

Bias exactness: q-bias applied to q (corr tile / bulk fix); k-bias dropped
(softmax is invariant to per-row constants); v-bias added post-attention
(softmax rows sum to 1).  All weight-derived constants precomputed on host.
"""

import numpy as np
from contextlib import ExitStack

import ml_dtypes

R = 128
CIN = 256
COUT = 256
HW = R * R
NCORES = 8
USE_F32R = True          # fast fp32 path for the resize matmul
USE_STRIDE0_FIX = True   # bulk q-bias fix via stride-0 broadcast AP
LINEARIZE = False        # total-order scheduling (deadlock workaround if needed)

_CACHE = {}


def _build():
    try:
        import concourse.bass as bass
    except ImportError:
        import sys
        for p in ("/opt/trn_rl_repo", "/root/.axon_site/_ro/trn_rl_repo"):
            if p not in sys.path:
                sys.path.append(p)
        import concourse.bass as bass
    import concourse.tile as tile
    from concourse import bacc, mybir

    BF = mybir.dt.bfloat16
    F32 = mybir.dt.float32
    FX = mybir.dt.float32r if USE_F32R else mybir.dt.float32
    AF = mybir.ActivationFunctionType
    ALU = mybir.AluOpType
    AX = mybir.AxisListType
    ts = bass.ts

    nc = bacc.Bacc("TRN2", target_bir_lowering=False, debug=False)

    def din(name, shape, dt):
        return nc.dram_tensor(name, shape, dt, kind="ExternalInput").ap()

    x = din("x", [CIN, HW], FX)
    wrT = din("wrT", [CIN, R], FX)
    br = din("br", [R, 1], F32)
    wqkv_w = din("wqkv_w", [R, 3 * R], BF)   # [WqwT|WkwT|WvwT]
    corr_w = din("corr_w", [R, 3 * R], F32)  # [bqw bcast | 0 | 0]
    wqkv_h = din("wqkv_h", [R, 3 * R], BF)
    corr_h = din("corr_h", [R, 3 * R], F32)  # [bqh bcast | 0 | 0]
    bvw_rep = din("bvw_rep", [R, R], F32)    # bvw[o] replicated down partitions
    bvh_rep = din("bvh_rep", [R, R], F32)
    woT = din("woT", [R, COUT], BF)
    bo2 = din("bo2", [R, 2], F32)
    ident = din("ident", [R, R], BF)
    y = nc.dram_tensor("y", [COUT, HW], F32, kind="ExternalOutput").ap()

    with tile.TileContext(nc, linearize=LINEARIZE) as tc, ExitStack() as ctx:
        const = ctx.enter_context(tc.tile_pool(name="const", bufs=1))

        _cn = [0]

        def cload(ap, dt):
            _cn[0] += 1
            t = const.tile(list(ap.shape), dt, tag=f"c{_cn[0]}_{ap.tensor.name}")
            nc.sync.dma_start(t[:], ap)
            return t

        wrT_a = cload(wrT[0:R, :], FX)
        wrT_b = cload(wrT[R:CIN, :], FX)
        br_sb = cload(br, F32)
        wqkvw_sb = cload(wqkv_w, BF)
        corrw_sb = cload(corr_w, F32)
        wqkvh_sb = cload(wqkv_h, BF)
        corrh_sb = cload(corr_h, F32)
        bvwr_sb = cload(bvw_rep, F32)
        bvhr_sb = cload(bvh_rep, F32)
        woT_sb = cload(woT, BF)
        bo2_sb = cload(bo2, F32)
        id_sb = cload(ident, BF)

        psum = ctx.enter_context(tc.tile_pool(name="ps", bufs=4, space="PSUM"))
        psbf = ctx.enter_context(tc.tile_pool(name="psb", bufs=4, space="PSUM"))

        # ---------- Stage A: resize -> Z [r, (h,w)] bf16 ----------
        qkvp = ctx.enter_context(tc.tile_pool(name="qkvp", bufs=1))
        qkvs = qkvp.tile([R, 3 * HW], BF, tag="qkvs")  # [q|k|v], each [s, o*128+c]
        OQ, OK, OV = 0, HW, 2 * HW

        def qkv_phase(src, wq_sb, corr_sb):
            src_v = src[:].rearrange("p (t c) -> p t c", c=R)
            qkvs_v = qkvs[:].rearrange("p (s o c) -> p s o c", s=3, c=R)
            for c in range(R):
                pq = psum.tile([R, 3 * R], F32, tag="mm")
                nc.tensor.matmul(pq[:], src_v[:, :, c], wq_sb[:],
                                 start=True, stop=True)
                pq_v = pq[:].rearrange("p (s o) -> p s o", s=3)
                # q section on DVE with bias add; k,v on ACT (exact: k-bias
                # dropped by softmax invariance, v-bias deferred).
                nc.vector.tensor_tensor(qkvs_v[:, 0, :, c], pq_v[:, 0, :],
                                        corr_sb[:, 0:R], op=ALU.add)
                nc.scalar.activation(qkvs_v[:, 1:3, :, c], pq_v[:, 1:3, :],
                                     AF.Identity)

        def attn_phase(bvr_sb, out_tile, height):
            with tc.tile_pool(name=f"att{int(height)}", bufs=6) as att:
                for og in range(32):
                    pS = psum.tile([R, 512], F32, tag="mm")
                    for i in range(4):
                        o = og * 4 + i
                        nc.tensor.matmul(pS[:, ts(i, R)],
                                         qkvs[:, OQ + o * R:OQ + (o + 1) * R],
                                         qkvs[:, OK + o * R:OK + (o + 1) * R],
                                         start=True, stop=True)
                    e4 = att.tile([R, 512], BF, tag="e4")
                    nc.scalar.activation(e4[:], pS[:], AF.Exp)
                    sm = att.tile([R, 4], F32, tag="sm")
                    nc.vector.tensor_reduce(
                        sm[:], e4[:].rearrange("p (i u) -> p i u", u=R),
                        axis=AX.X, op=ALU.add)
                    rc = att.tile([R, 4], F32, tag="rc")
                    nc.vector.reciprocal(rc[:], sm[:])
                    pn4 = att.tile([R, 512], BF, tag="pn4",
                                   name=f"pn4_{height}_{og}") if not height else None
                    if not height:
                        for i in range(4):
                            nc.vector.tensor_scalar_mul(
                                pn4[:, ts(i, R)], e4[:, ts(i, R)],
                                rc[:, i:i + 1])
                    psrc = e4 if height else pn4
                    pvT = psbf.tile([R, 1024], BF, tag="tp")
                    for i in range(4):
                        o = og * 4 + i
                        nc.tensor.transpose(pvT[:, ts(i, R)], psrc[:, ts(i, R)],
                                            id_sb[:])
                        nc.tensor.transpose(pvT[:, ts(4 + i, R)],
                                            qkvs[:, OV + o * R:OV + (o + 1) * R],
                                            id_sb[:])
                    pvs = att.tile([R, 1024], BF, tag="pvs")
                    if og % 2 == 0:
                        nc.vector.tensor_copy(pvs[:], pvT[:])
                    else:
                        nc.scalar.activation(pvs[:], pvT[:], AF.Identity)
                    pnT = pvs[:, 0:512]
                    vTs = pvs[:, 512:1024]
                    pO = psum.tile([R, 512], F32, tag="mm")
                    for i in range(4):
                        if height:
                            nc.tensor.matmul(pO[:, ts(i, R)], pnT[:, ts(i, R)],
                                             vTs[:, ts(i, R)],
                                             start=True, stop=True)
                        else:
                            nc.tensor.matmul(pO[:, ts(i, R)], vTs[:, ts(i, R)],
                                             pnT[:, ts(i, R)],
                                             start=True, stop=True)
                    for i in range(4):
                        o = og * 4 + i
                        if height:
                            od = out_tile[o // 16]
                            osl = ts(o % 16, R)
                            if i % 2 == 0:
                                nc.scalar.activation(
                                    od[:, osl], pO[:, ts(i, R)],
                                    AF.Identity, bias=bvr_sb[:, o:o + 1],
                                    scale=rc[:, i:i + 1])
                            else:
                                nc.vector.tensor_scalar(
                                    od[:, osl], pO[:, ts(i, R)],
                                    rc[:, i:i + 1], bvr_sb[:, o:o + 1],
                                    op0=ALU.mult, op1=ALU.add)
                        else:
                            if i % 2 == 0:
                                nc.scalar.activation(
                                    out_tile[:, ts(o, R)], pO[:, ts(i, R)],
                                    AF.Identity, bias=bvr_sb[:, o:o + 1])
                            else:
                                nc.vector.tensor_scalar_add(
                                    out_tile[:, ts(o, R)], pO[:, ts(i, R)],
                                    bvr_sb[:, o:o + 1])

        with tc.tile_pool(name="pAB", bufs=1) as pAB:
            zbf = pAB.tile([R, HW], BF, tag="zbf")
            with tc.tile_pool(name="xin", bufs=4) as xin:
                for j in range(32):
                    sl = ts(j, 512)
                    xa = xin.tile([R, 512], FX, tag="xa")
                    nc.sync.dma_start(xa[:], x[0:R, sl])
                    xb = xin.tile([R, 512], FX, tag="xb")
                    nc.sync.dma_start(xb[:], x[R:CIN, sl])
                    ps = psum.tile([R, 512], F32, tag="mm")
                    nc.tensor.matmul(ps[:], wrT_a[:], xa[:], start=True,
                                     stop=False)
                    nc.tensor.matmul(ps[:], wrT_b[:], xb[:], start=False,
                                     stop=True)
                    if j % 2 == 0:
                        nc.scalar.activation(zbf[:, sl], ps[:], AF.Identity,
                                             bias=br_sb[:])
                    else:
                        nc.vector.tensor_scalar_add(zbf[:, sl], ps[:], br_sb[:])

            # ---------- Stage B: Zp [h, w*128+r] bf16 ----------
            zp = pAB.tile([R, HW], BF, tag="zp")
            zbf_v = zbf[:].rearrange("p (h w) -> p h w", w=R)
            for g in range(32):
                pt = psbf.tile([R, 512], BF, tag="tp")
                for i in range(4):
                    nc.tensor.transpose(pt[:, ts(i, R)], zbf_v[:, :, g * 4 + i],
                                        id_sb[:])
                if g % 2 == 0:
                    nc.vector.tensor_copy(zp[:, ts(g, 512)], pt[:])
                else:
                    nc.scalar.activation(zp[:, ts(g, 512)], pt[:], AF.Identity)

            qkv_phase(zp, wqkvw_sb, corrw_sb)

        with tc.tile_pool(name="pW", bufs=1) as pW:
            wp = pW.tile([R, HW], BF, tag="wp")
            attn_phase(bvwr_sb, wp, height=False)
            qkv_phase(wp, wqkvh_sb, corrh_sb)

            with tc.tile_pool(name="pH", bufs=1) as pH:
                h2s = [pH.tile([R, HW // 8], BF, tag="h2", bufs=8,
                               name=f"h2_{i}") for i in range(8)]
                attn_phase(bvhr_sb, h2s, height=True)

                # ---------- restore (inside pH scope) ----------
                with tc.tile_pool(name="yout", bufs=6) as yp:
                    for j in range(32):
                        sl = ts(j, 512)
                        for half in range(2):
                            pY = psum.tile([R, 512], F32, tag="mm")
                            nc.tensor.matmul(pY[:], woT_sb[:, ts(half, R)],
                                             h2s[j // 4][:, ts(j % 4, 512)],
                                             start=True, stop=True)
                            yt = yp.tile([R, 512], F32, tag="yt")
                            if (j + half) % 2 == 0:
                                nc.scalar.activation(
                                    yt[:], pY[:], AF.Identity,
                                    bias=bo2_sb[:, half:half + 1])
                            else:
                                nc.vector.tensor_scalar_add(
                                    yt[:], pY[:], bo2_sb[:, half:half + 1])
                            nc.sync.dma_start(y[half * R:(half + 1) * R, sl],
                                              yt[:])

    nc.compile()
    return nc


def _get_nc():
    if "nc" not in _CACHE:
        _CACHE["nc"] = _build()
    return _CACHE["nc"]


def _host_inputs(inputs):
    bf16 = ml_dtypes.bfloat16
    f32 = np.float32
    g = {k: np.asarray(v) for k, v in inputs.items()}
    Wr, br = g["Wr"], g["br"]
    Wqw, bqw, Wkw, Wvw, bvw = g["Wqw"], g["bqw"], g["Wkw"], g["Wvw"], g["bvw"]
    Wqh, bqh, Wkh, Wvh, bvh = g["Wqh"], g["bqh"], g["Wkh"], g["Wvh"], g["bvh"]
    Wo, bo = g["Wo"], g["bo"]

    com = dict(
        wrT=np.ascontiguousarray(Wr.T).astype(f32),
        br=np.ascontiguousarray(br[:, None]).astype(f32),
        wqkv_w=np.ascontiguousarray(
            np.concatenate([Wqw.T, Wkw.T, Wvw.T], 1)).astype(bf16),
        corr_w=np.ascontiguousarray(np.concatenate(
            [np.tile(bqw[None, :], (R, 1)), np.zeros((R, 2 * R))], 1)).astype(f32),
        wqkv_h=np.ascontiguousarray(
            np.concatenate([Wqh.T, Wkh.T, Wvh.T], 1)).astype(bf16),
        corr_h=np.ascontiguousarray(np.concatenate(
            [np.tile(bqh[None, :], (R, 1)), np.zeros((R, 2 * R))], 1)).astype(f32),
        bvw_rep=np.ascontiguousarray(np.tile(bvw[None, :], (R, 1))).astype(f32),
        bvh_rep=np.ascontiguousarray(np.tile(bvh[None, :], (R, 1))).astype(f32),
        woT=np.ascontiguousarray(Wo.T).astype(bf16),
        bo2=np.ascontiguousarray(bo.reshape(2, R).T).astype(f32),
        ident=np.eye(R).astype(bf16),
    )
    xs = g["x"].astype(f32)
    in_maps = []
    for b in range(NCORES):
        m = dict(com)
        m["x"] = np.ascontiguousarray(xs[b].reshape(CIN, HW))
        in_maps.append(m)
    return in_maps


def run(inputs, trace=False, **kw):
    try:
        from concourse.bass_utils import run_bass_kernel_spmd
    except ImportError:
        import sys
        for p in ("/opt/trn_rl_repo", "/root/.axon_site/_ro/trn_rl_repo"):
            if p not in sys.path:
                sys.path.append(p)
        from concourse.bass_utils import run_bass_kernel_spmd
    nc = _get_nc()
    in_maps = _host_inputs(inputs)
    res = run_bass_kernel_spmd(nc, in_maps, list(range(NCORES)),
                               trace=trace, **kw)
    out = np.stack([res.results[b]["y"].reshape(COUT, R, R).transpose(0, 2, 1)
                    for b in range(NCORES)], 0)
    return out.astype(np.float32), res


def kernel(**inputs):
    out, _ = run(inputs, trace=False)
    return out



# revision 4
# speedup vs baseline: 1.3632x; 1.3632x over previous
"""CSAA (criss-cross axial attention) Trainium2 kernel, v2.

Sharding: pure data parallel — batch element b -> NeuronCore b (B=8 on 8 cores).

Per-core pipeline (R=128, CIN=COUT=256, H=W=128, HW=16384), per batch element:
  xT [c, (w,h)] bf16 (host-transposed)
    --stage A (per-w stationary mm)-->  Zp [h, (w,r)] bf16          (+br)
  width branch:  qkv (per-r stationary mm) -> qkvs [w, s,o,c]       (+corr_w)
                 S^T-form attention -> Wp [w, (h,r)] bf16
  height branch: qkv -> qkvs [h, s,o',c]                            (+corr_h)
                 S^T-form attention -> H2 [r, (w,h)] streamed
  restore:       Y [co, (w,h)] bf16 -> host f32 + transpose + bo

Attention (S^T form, softmax over the PARTITION axis):
  S^T[u,r] = matmul(lhsT=k_blk, rhs=q_blk); E = exp(S^T) bf16
  width:  sums = gpsimd.partition_all_reduce(E) (replicated f32);
          OutT[w,r] = matmul(lhsT=V^T_blk, rhs=E_blk); out = OutT / sums
  height: sums_T[r,1] = matmul(lhsT=E_blk, rhs=ones); rcT = 1/sums_T;
          Out[r,h] = matmul(lhsT=E_blk, rhs=V^T_blk); out = Out * rcT[r]

Bias exactness: q-bias via corr tile (affects softmax); k-bias dropped
(u-independent shift, softmax-invariant in S^T form); v-bias added at the
qkv copy via corr v-section (rides through attention exactly since softmax
rows sum to 1); bo added on host.
"""

import numpy as np
from contextlib import ExitStack

import ml_dtypes

R = 128
CIN = 256
COUT = 256
HW = R * R
NCORES = 8
LINEARIZE = False
_CACHE = {}


def _build():
    try:
        import concourse.bass as bass
    except ImportError:
        import sys
        for p in ("/opt/trn_rl_repo", "/root/.axon_site/_ro/trn_rl_repo"):
            if p not in sys.path:
                sys.path.append(p)
        import concourse.bass as bass
    import concourse.tile as tile
    from concourse import bacc, mybir, bass_isa

    BF = mybir.dt.bfloat16
    F32 = mybir.dt.float32
    AF = mybir.ActivationFunctionType
    ALU = mybir.AluOpType
    RED = bass_isa.ReduceOp
    ts = bass.ts

    nc = bacc.Bacc("TRN2", target_bir_lowering=False, debug=False)

    def din(name, shape, dt):
        return nc.dram_tensor(name, shape, dt, kind="ExternalInput").ap()

    xT = din("xT", [CIN, HW], BF)        # [c, w*128+h]
    wrT = din("wrT", [CIN, R], BF)
    brp = din("brp", [R, 512], F32)      # br[r] tiled x4, replicated partitions
    wqkv_w = din("wqkv_w", [R, 3 * R], BF)   # [WqwT|WkwT|WvwT]
    corr_w = din("corr_w", [R, 2 * R], F32)  # [bqw tile | bvw tile]
    wqkv_h = din("wqkv_h", [R, 3 * R], BF)
    corr_h = din("corr_h", [R, 2 * R], F32)  # [bqh tile | bvh tile]
    woT = din("woT", [R, COUT], BF)
    ident = din("ident", [R, R], BF)
    ones = din("ones", [R, 1], BF)
    y = nc.dram_tensor("y", [COUT, HW], BF, kind="ExternalOutput").ap()

    OQ, OK, OV = 0, HW, 2 * HW

    with tile.TileContext(nc, linearize=LINEARIZE) as tc, ExitStack() as ctx:
        const = ctx.enter_context(tc.tile_pool(name="const", bufs=1))

        _cn = [0]

        def cload(ap, dt):
            _cn[0] += 1
            t = const.tile(list(ap.shape), dt, tag=f"c{_cn[0]}_{ap.tensor.name}")
            nc.sync.dma_start(t[:], ap)
            return t

        wrT_a = cload(wrT[0:R, :], BF)
        wrT_b = cload(wrT[R:CIN, :], BF)
        brp_sb = cload(brp, F32)
        wqkvw_sb = cload(wqkv_w, BF)
        corrw_sb = cload(corr_w, F32)
        wqkvh_sb = cload(wqkv_h, BF)
        corrh_sb = cload(corr_h, F32)
        woT_sb = cload(woT, BF)
        id_sb = cload(ident, BF)
        ones_sb = cload(ones, BF)

        # rotating DVE/Pool dispatch for tensor_tensor ops
        _rr = [0]

        def rot_tt(dst, src, opnd, op, pool_every=3):
            _rr[0] += 1
            if _rr[0] % pool_every == 0:
                nc.gpsimd.tensor_tensor(dst, src, opnd, op=op)
            else:
                nc.vector.tensor_tensor(dst, src, opnd, op=op)

        _rc = [0]

        def rot_copy(dst, src, seq="vs"):
            e = seq[_rc[0] % len(seq)]
            _rc[0] += 1
            if e == "v":
                nc.vector.tensor_copy(dst, src)
            elif e == "s":
                nc.scalar.activation(dst, src, AF.Identity)
            else:
                nc.gpsimd.tensor_copy(dst, src)

        qkvp = ctx.enter_context(tc.tile_pool(name="qkvp", bufs=1))
        qkvs = qkvp.tile([R, 3 * HW], BF, tag="qkvs")  # [q|k|v], [p, o*128+c]
        qkvs_c = qkvs[:].rearrange("p (s o c) -> p c s o", s=3, c=R)

        def qkv_phase(src, wq_sb, corr_sb):
            # src [p, (t, c)] : contract over p, per-c stationary slice
            src_v = src[:].rearrange("p (t c) -> p c t", c=R)
            corr_b = corr_sb[:].rearrange("p (s o) -> p s o", s=2)
            corr_b = corr_b[:, None, :, :].broadcast_to([R, 4, 2, R])
            with tc.tile_pool(name="pQ", bufs=2, space="PSUM") as pQ:
                for cg in range(32):
                    pq = pQ.tile([R, 2048], F32, tag="pq")
                    for i in range(4):
                        c = cg * 4 + i
                        nc.tensor.matmul(pq[:, i * 512:i * 512 + 384],
                                         src_v[:, c, :], wq_sb[:],
                                         start=True, stop=True)
                    pq_v = pq[:].rearrange("p (c x) -> p c x", c=4)
                    pq_v = pq_v[:, :, 0:384].rearrange("p c (s o) -> p c s o",
                                                       s=3)
                    # q+v sections: corr add on DVE/Pool
                    dst_qv = qkvs_c[:, cg * 4:cg * 4 + 4, 0::2, :]
                    rot_tt(dst_qv, pq_v[:, :, 0::2, :], corr_b, ALU.add)
                    # k section: pure copy on ACT
                    nc.scalar.activation(qkvs_c[:, cg * 4:cg * 4 + 4, 1, :],
                                         pq_v[:, :, 1, :], AF.Identity)

        def attn_w_phase(wp):
            with tc.tile_pool(name="aw", bufs=3) as aw, \
                 tc.tile_pool(name="pmm", bufs=4, space="PSUM") as pmm, \
                 tc.tile_pool(name="pvt", bufs=2, space="PSUM") as pvt:
                for og in range(32):
                    pS = pmm.tile([R, 512], F32, tag="mm")
                    for i in range(4):
                        o = og * 4 + i
                        nc.tensor.matmul(pS[:, ts(i, R)],
                                         qkvs[:, OK + o * R:OK + (o + 1) * R],
                                         qkvs[:, OQ + o * R:OQ + (o + 1) * R],
                                         start=True, stop=True)
                    e4 = aw.tile([R, 512], BF, tag="e4")
                    nc.scalar.activation(e4[:], pS[:], AF.Exp)
                    sums = aw.tile([R, 512], F32, tag="sums")
                    nc.gpsimd.partition_all_reduce(sums[:], e4[:], R, RED.add)
                    pvT = pvt.tile([R, 512], BF, tag="tp")
                    for i in range(4):
                        o = og * 4 + i
                        nc.tensor.transpose(pvT[:, ts(i, R)],
                                            qkvs[:, OV + o * R:OV + (o + 1) * R],
                                            id_sb[:])
                    vts = aw.tile([R, 512], BF, tag="vts")
                    rot_copy(vts[:], pvT[:], seq="vs")
                    pO = pmm.tile([R, 512], F32, tag="mm")
                    for i in range(4):
                        nc.tensor.matmul(pO[:, ts(i, R)], vts[:, ts(i, R)],
                                         e4[:, ts(i, R)], start=True, stop=True)
                    rot_tt(wp[:, ts(og, 512)], pO[:], sums[:], ALU.divide,
                           pool_every=4)

        def attn_h_restore(ybf_pool, h2_pool):
            with tc.tile_pool(name="ah", bufs=3) as ah, \
                 tc.tile_pool(name="pmm", bufs=4, space="PSUM") as pmm, \
                 tc.tile_pool(name="pvt", bufs=2, space="PSUM") as pvt:
                for og in range(32):
                    pS = pmm.tile([R, 512], F32, tag="mm")
                    for i in range(4):
                        o = og * 4 + i
                        nc.tensor.matmul(pS[:, ts(i, R)],
                                         qkvs[:, OK + o * R:OK + (o + 1) * R],
                                         qkvs[:, OQ + o * R:OQ + (o + 1) * R],
                                         start=True, stop=True)
                    e4 = ah.tile([R, 512], BF, tag="e4")
                    nc.scalar.activation(e4[:], pS[:], AF.Exp)
                    pvT = pvt.tile([R, 512], BF, tag="tp")
                    for i in range(4):
                        o = og * 4 + i
                        nc.tensor.transpose(pvT[:, ts(i, R)],
                                            qkvs[:, OV + o * R:OV + (o + 1) * R],
                                            id_sb[:])
                    vts = ah.tile([R, 512], BF, tag="vts")
                    rot_copy(vts[:], pvT[:], seq="vs")
                    # column sums: reuse pS (exp already consumed it)
                    for i in range(4):
                        nc.tensor.matmul(pS[:, i:i + 1], e4[:, ts(i, R)],
                                         ones_sb[:], start=True, stop=True)
                    rcT = ah.tile([R, 4], F32, tag="rcT")
                    nc.vector.reciprocal(rcT[:], pS[:, 0:4])
                    pO = pmm.tile([R, 512], F32, tag="mm")
                    for i in range(4):
                        nc.tensor.matmul(pO[:, ts(i, R)], e4[:, ts(i, R)],
                                         vts[:, ts(i, R)], start=True,
                                         stop=True)
                    h2 = h2_pool.tile([R, 512], BF, tag="h2")
                    for i in range(4):
                        if i % 2 == 0:
                            nc.scalar.activation(h2[:, ts(i, R)],
                                                 pO[:, ts(i, R)], AF.Identity,
                                                 scale=rcT[:, i:i + 1])
                        else:
                            nc.vector.tensor_scalar_mul(
                                h2[:, ts(i, R)], pO[:, ts(i, R)],
                                rcT[:, i:i + 1])
                    # restore: y chunk for this og (bo added on host)
                    for half in range(2):
                        pY = pmm.tile([R, 512], F32, tag="mm")
                        nc.tensor.matmul(pY[:], woT_sb[:, ts(half, R)],
                                         h2[:], start=True, stop=True)
                        yt = ybf_pool.tile([R, 512], BF, tag="yt")
                        rot_copy(yt[:], pY[:], seq="svg")
                        nc.sync.dma_start(y[half * R:(half + 1) * R,
                                            ts(og, 512)], yt[:])

        # ---------- Stage A + width qkv (zp scoped) ----------
        with tc.tile_pool(name="pAB", bufs=1) as pAB:
            zp = pAB.tile([R, HW], BF, tag="zp")  # [h, w*128+r]
            with tc.tile_pool(name="xin", bufs=4) as xin, \
                 tc.tile_pool(name="pA", bufs=3, space="PSUM") as pA:
                for ch in range(8):          # 16 w-blocks per chunk
                    xa = xin.tile([R, 2048], BF, tag="xa")
                    nc.sync.dma_start(xa[:], xT[0:R, ts(ch, 2048)])
                    xb = xin.tile([R, 2048], BF, tag="xb")
                    nc.sync.dma_start(xb[:], xT[R:CIN, ts(ch, 2048)])
                    for wg in range(4):      # 4 w per psum tile
                        pa = pA.tile([R, 512], F32, tag="pa")
                        for i in range(4):
                            wl = wg * 4 + i  # w within chunk
                            nc.tensor.matmul(pa[:, ts(i, R)],
                                             xa[:, ts(wl, R)], wrT_a[:],
                                             start=True, stop=False)
                            nc.tensor.matmul(pa[:, ts(i, R)],
                                             xb[:, ts(wl, R)], wrT_b[:],
                                             start=False, stop=True)
                        g = ch * 4 + wg
                        rot_tt(zp[:, ts(g, 512)], pa[:], brp_sb[:], ALU.add)

            qkv_phase(zp, wqkvw_sb, corrw_sb)

        # ---------- width attention -> wp, height qkv ----------
        with tc.tile_pool(name="pW", bufs=1) as pW:
            wp = pW.tile([R, HW], BF, tag="wp")  # [w, o*128+r]
            attn_w_phase(wp)
            qkv_phase(wp, wqkvh_sb, corrh_sb)

        # ---------- height attention + restore (streamed) ----------
        with tc.tile_pool(name="h2p", bufs=4) as h2p, \
             tc.tile_pool(name="yout", bufs=6) as yp:
            attn_h_restore(yp, h2p)

    nc.compile()
    return nc


def _get_nc():
    if "nc" not in _CACHE:
        _CACHE["nc"] = _build()
    return _CACHE["nc"]


def _host_inputs(inputs):
    bf16 = ml_dtypes.bfloat16
    f32 = np.float32
    g = {k: np.asarray(v) for k, v in inputs.items()}
    Wr, br = g["Wr"], g["br"]
    Wqw, bqw, Wkw, Wvw, bvw = g["Wqw"], g["bqw"], g["Wkw"], g["Wvw"], g["bvw"]
    Wqh, bqh, Wkh, Wvh, bvh = g["Wqh"], g["bqh"], g["Wkh"], g["Wvh"], g["bvh"]
    Wo = g["Wo"]

    def tile_row(v):
        return np.tile(np.asarray(v)[None, :], (R, 1))

    com = dict(
        wrT=np.ascontiguousarray(Wr.T).astype(bf16),
        brp=np.ascontiguousarray(np.tile(np.asarray(br)[None, :], (R, 4))
                                 ).astype(f32),
        wqkv_w=np.ascontiguousarray(
            np.concatenate([Wqw.T, Wkw.T, Wvw.T], 1)).astype(bf16),
        corr_w=np.ascontiguousarray(np.concatenate(
            [tile_row(bqw), tile_row(bvw)], 1)).astype(f32),
        wqkv_h=np.ascontiguousarray(
            np.concatenate([Wqh.T, Wkh.T, Wvh.T], 1)).astype(bf16),
        corr_h=np.ascontiguousarray(np.concatenate(
            [tile_row(bqh), tile_row(bvh)], 1)).astype(f32),
        woT=np.ascontiguousarray(Wo.T).astype(bf16),
        ident=np.eye(R).astype(bf16),
        ones=np.ones((R, 1)).astype(bf16),
    )
    xs = np.asarray(g["x"])
    in_maps = []
    for b in range(NCORES):
        m = dict(com)
        m["xT"] = np.ascontiguousarray(
            xs[b].transpose(0, 2, 1).reshape(CIN, HW)).astype(bf16)
        in_maps.append(m)
    return in_maps


def run(inputs, trace=False, **kw):
    try:
        from concourse.bass_utils import run_bass_kernel_spmd
    except ImportError:
        import sys
        for p in ("/opt/trn_rl_repo", "/root/.axon_site/_ro/trn_rl_repo"):
            if p not in sys.path:
                sys.path.append(p)
        from concourse.bass_utils import run_bass_kernel_spmd
    nc = _get_nc()
    in_maps = _host_inputs(inputs)
    res = run_bass_kernel_spmd(nc, in_maps, list(range(NCORES)),
                               trace=trace, **kw)
    bo = np.asarray(inputs["bo"], dtype=np.float32)
    out = np.stack([np.asarray(res.results[b]["y"], dtype=np.float32)
                    .reshape(COUT, R, R).transpose(0, 2, 1)
                    for b in range(NCORES)], 0)
    out += bo[None, :, None, None]
    return out.astype(np.float32), res


def kernel(**inputs):
    out, _ = run(inputs, trace=False)
    return out
